# revision 5
# baseline (speedup 1.0000x reference)
"""GCN sampling kernel v4: unique-h gathering + SEL-matmul layer-1 pooling.

vs v3: computes only the ~4826 UNIQUE layer-1 h-rows per core (38 blocks
-> 380 gather instructions instead of 400; each costs a fixed ~1037ns of
Pool-engine SWDGE time, the kernel's hard bottleneck). Layer-1 mean-pooling
can no longer use the fixed-stride trick (no duplication), so it is done on
the tensor engine: host-built selector matrices S[h_slot, dst] (entry =
edge multiplicity) contracted against the per-block h1 slices, accumulated
in PSUM f32 across all blocks. Output is produced transposed [64, 512] and
transposed back on the host.
"""

import sys

sys.path.insert(0, "/opt/trn_rl_repo")

from contextlib import ExitStack

import numpy as np
import ml_dtypes

N0, N1, N2 = 409600, 40960, 4096
F = 10
IN_F, HID, NCLS = 512, 256, 64
NC_N = 8
DST_PC = N2 // NC_N         # 512 dst nodes per core
BLK = 128

_BUILT = {}


def _legalize_waits(bir: bytes) -> bytes:
    import orjson

    j = orjson.loads(bir)
    n_new = 0
    for fn in j["functions"]:
        for bb in fn["blocks"]:
            insts = bb["instructions"]
            out = []
            for inst in insts:
                si = inst.get("sync_info")
                waits = si.get("on_wait") if si else None
                if waits and len(waits) > 1:
                    for w in waits[:-1]:
                        n_new += 1
                        out.append({
                            "debug": inst.get("debug", 0),
                            "engine": inst["engine"],
                            "ins": [],
                            "name": f"{inst['name']}_esw{n_new}",
                            "opcode": "EventSemaphore",
                            "outs": [],
                            "sync_info": {"on_update": [], "on_wait": [w]},
                        })
                    si["on_wait"] = [waits[-1]]
                out.append(inst)
            bb["instructions"] = out
    return orjson.dumps(j)


def _install_patch():
    import concourse.bass as bass

    if getattr(bass.Bass, "_gcn_wait_patch", False):
        return
    orig = bass.Bass.to_json_bytes

    def to_json_bytes(self, *a, **kw):
        return _legalize_waits(orig(self, *a, **kw))

    bass.Bass.to_json_bytes = to_json_bytes
    bass.Bass._gcn_wait_patch = True


def build_nc(nb1):
    _install_patch()
    import concourse.bass as bass
    import concourse.tile as tile
    from concourse import mybir
    from concourse.masks import make_identity

    f32 = mybir.dt.float32
    bf16 = mybir.dt.bfloat16
    nc = bass.Bass("TRN2", target_bir_lowering=False, debug=False,
                   num_devices=NC_N, num_swdge_queues=4)

    feat = nc.dram_tensor("feat", [N0, IN_F], bf16, kind="ExternalInput")
    w1 = nc.dram_tensor("w1", [IN_F, HID], bf16, kind="ExternalInput")
    w2 = nc.dram_tensor("w2", [HID, NCLS], bf16, kind="ExternalInput")
    idx = nc.dram_tensor("idx", [BLK, nb1 * F], mybir.dt.int32,
                         kind="ExternalInput")
    sel = nc.dram_tensor("sel", [BLK, nb1 * 4 * 128], bf16,
                         kind="ExternalInput")
    outT = nc.dram_tensor("outT", [NCLS, DST_PC], f32, kind="ExternalOutput")

    with tile.TileContext(nc) as tc, ExitStack() as ctx:
        consts = ctx.enter_context(tc.tile_pool(name="consts", bufs=1))
        gpool = ctx.enter_context(tc.tile_pool(name="gather", bufs=6))
        hpool = ctx.enter_context(tc.tile_pool(name="hhTc", bufs=2))
        h1pool = ctx.enter_context(tc.tile_pool(name="h1", bufs=1))
        tpool = ctx.enter_context(tc.tile_pool(name="temps", bufs=2))
        ps_tr = ctx.enter_context(tc.tile_pool(name="ps_tr", bufs=2,
                                               space="PSUM"))
        ps_w1 = ctx.enter_context(tc.tile_pool(name="ps_w1", bufs=2,
                                               space="PSUM"))
        ps_l1 = ctx.enter_context(tc.tile_pool(name="ps_l1", bufs=1,
                                               space="PSUM"))

        idx_t = consts.tile([BLK, nb1 * F], mybir.dt.int32)
        nc.sync.dma_start(idx_t[:], idx.ap())
        sel_t = consts.tile([BLK, nb1 * 4 * 128], bf16)
        # w1t: col fc*256+h = w1[fc*128+p, h]
        w1t = consts.tile([128, 4 * HID], bf16)
        nc.sync.dma_start(w1t[:].rearrange("p (f h) -> p f h", f=4),
                          w1.ap().rearrange("(f p) h -> p f h", f=4))
        # w2t: col c*64+n = w2[c*128+p, n]
        w2t = consts.tile([128, 2 * NCLS], bf16)
        nc.sync.dma_start(w2t[:].rearrange("p (c n) -> p c n", c=2),
                          w2.ap().rearrange("(c p) n -> p c n", c=2))
        ident = consts.tile([128, 128], bf16)
        make_identity(nc, ident[:])

        # h1 table: [128 slot-part, nb1*256] bf16 (slot s = [s%128, s//128])
        h1t = h1pool.tile([128, nb1 * HID], bf16, name="h1t")
        # layer-1 accumulators: [128 dst, 256 hid] f32 per dst-block
        pl1 = [ps_l1.tile([128, HID], f32, tag=f"pl1_{rb}", space="PSUM",
                          name=f"pl1_{rb}") for rb in range(4)]

        sizes = [4] * (nb1 // 4) + ([] if nb1 % 4 == 0 else [nb1 % 4])
        if sizes and sizes[-1] < 3 and len(sizes) >= 2:
            # avoid a trailing <3-chain wave: steal one block from the
            # previous wave so the last two waves have >=3 chains each
            take = 3 - sizes[-1]
            sizes[-2] -= take
            sizes[-1] += take
        waves = []
        st = 0
        for s in sizes:
            waves.append(list(range(st, st + s)))
            st += s
        for wave in waves:
            hss = [gpool.tile([BLK, IN_F], bf16, tag=f"hs{j}",
                              name=f"hs_{wave[0]}_{j}")
                   for j in range(len(wave))]
            for k in range(F):
                for j, b in enumerate(wave):
                    nc.gpsimd.indirect_dma_start(
                        out=hss[j][:], out_offset=None, in_=feat.ap(),
                        in_offset=bass.IndirectOffsetOnAxis(
                            ap=idx_t[:, b * F + k:b * F + k + 1], axis=0),
                        compute_op=(mybir.AluOpType.bypass if k == 0
                                    else mybir.AluOpType.add),
                    )
            for j, b in enumerate(wave):
                c0, c1 = b * 4 * 128, (b + 1) * 4 * 128
                nc.sync.dma_start(sel_t[:, c0:c1], sel.ap()[:, c0:c1])
                # transpose hs -> 4 chunks [128 f, 128 g], then W1 per block
                pw = ps_w1.tile([128, HID], f32, tag="pw", space="PSUM",
                                name=f"pw_{b}")
                for fc in range(4):
                    ptr = ps_tr.tile([128, 128], f32, tag="ptr", space="PSUM",
                                     name=f"ptr_{b}_{fc}")
                    nc.tensor.matmul(ptr[:],
                                     lhsT=hss[j][:, fc * 128:(fc + 1) * 128],
                                     rhs=ident[:], start=True, stop=True,
                                     skip_group_check=True)
                    hc_sb = hpool.tile([128, 128], bf16, tag=f"hc{fc}",
                                       name=f"hc_{b}_{fc}")
                    if fc % 2 == 0:
                        nc.vector.tensor_copy(hc_sb[:], ptr[:])
                    else:
                        nc.scalar.activation(
                            hc_sb[:], ptr[:],
                            mybir.ActivationFunctionType.Copy)
                    nc.tensor.matmul(pw[:], lhsT=hc_sb[:],
                                     rhs=w1t[:, fc * HID:(fc + 1) * HID],
                                     start=(fc == 0), stop=(fc == 3),
                                     skip_group_check=True)
                nc.scalar.activation(h1t[:, b * HID:(b + 1) * HID], pw[:],
                                     mybir.ActivationFunctionType.Relu)
                for rb in range(4):
                    nc.tensor.matmul(
                        pl1[rb][:],
                        lhsT=sel_t[:, (b * 4 + rb) * 128:
                                   (b * 4 + rb) * 128 + 128],
                        rhs=h1t[:, b * HID:(b + 1) * HID],
                        start=(b == 0), stop=(b == nb1 - 1),
                        skip_group_check=True)

        # tail: pooled2 [128 dst, 256] f32 per rb -> transpose -> W2 -> store
        for rb in range(4):
            p2 = tpool.tile([128, HID], bf16, tag="p2", name=f"p2_{rb}")
            nc.vector.tensor_copy(p2[:], pl1[rb][:])
            p2T = tpool.tile([128, 2 * 128], bf16, tag="p2T",
                             name=f"p2T_{rb}")
            for hc in range(2):
                ptr = ps_tr.tile([128, 128], f32, tag="ptr", space="PSUM",
                                 name=f"ptrT_{rb}_{hc}")
                nc.tensor.matmul(ptr[:], lhsT=p2[:, hc * 128:(hc + 1) * 128],
                                 rhs=ident[:], start=True, stop=True,
                                 skip_group_check=True)
                nc.vector.tensor_copy(p2T[:, hc * 128:(hc + 1) * 128],
                                      ptr[:])
            po = ps_w1.tile([128, HID], f32, tag="pw", space="PSUM",
                            name=f"po_{rb}")
            for hc in range(2):
                nc.tensor.matmul(
                    po[0:NCLS, 0:128],
                    lhsT=w2t[:, hc * NCLS:(hc + 1) * NCLS],
                    rhs=p2T[:, hc * 128:(hc + 1) * 128],
                    start=(hc == 0), stop=(hc == 1),
                    skip_group_check=True)
            osb = tpool.tile([NCLS, 128], f32, tag="osb", name=f"osb_{rb}")
            nc.vector.tensor_copy(osb[:], po[0:NCLS, 0:128])
            nc.sync.dma_start(outT.ap()[:, rb * 128:(rb + 1) * 128], osb[:])

    return nc


def _get_nc(nb1=38):
    if nb1 not in _BUILT:
        _BUILT[nb1] = build_nc(nb1)
    return _BUILT[nb1]


def _prep_core(src0, src1, core, nb1):
    """Per-core unique-h index table and layer-1 SEL matrix."""
    s1 = src1[core * DST_PC * F:(core + 1) * DST_PC * F].astype(np.int64)
    uniq = np.unique(s1)                      # sorted unique h-ids
    nu = len(uniq)
    nslots = nb1 * BLK
    assert nu <= nslots
    slots = np.full(nslots, uniq[-1], np.int64)
    slots[:nu] = uniq
    # gather indices: idx[p, b*F+k] = src0[slots[b*128+p]*F + k]
    G = src0[(slots[:, None] * F + np.arange(F)[None, :])]   # [nslots, F]
    idx = np.ascontiguousarray(
        G.reshape(nb1, BLK, F).transpose(1, 0, 2).reshape(BLK, nb1 * F)
    ).astype(np.int32)
    # SEL: S[p, (b*4+rb)*128 + j] = multiplicity of dst (rb*128+j)
    # referencing slot (b*128+p)
    e_slot = np.searchsorted(uniq, s1)        # [5120] slot of each edge
    e_dst = np.arange(DST_PC * F) // F
    S = np.zeros((nslots, DST_PC), np.float32)
    np.add.at(S, (e_slot, e_dst), 1.0)
    S4 = S.reshape(nb1, BLK, 4, 128).transpose(1, 0, 2, 3).reshape(
        BLK, nb1 * 4 * 128)
    return idx, np.ascontiguousarray(S4).astype(ml_dtypes.bfloat16)


def _run(inputs, trace=False, trace_kwargs=None):
    from concourse.bass_utils import run_bass_kernel_spmd

    featb = np.ascontiguousarray(
        np.asarray(inputs["features"], dtype=np.float32)
    ).astype(ml_dtypes.bfloat16)
    w1s = (np.ascontiguousarray(inputs["W1"], dtype=np.float32)
           / np.float32(F)).astype(ml_dtypes.bfloat16)
    w2s = (np.ascontiguousarray(inputs["W2"], dtype=np.float32)
           / np.float32(F)).astype(ml_dtypes.bfloat16)
    b1 = np.ascontiguousarray(inputs["b1"], dtype=np.float32)
    b2 = np.ascontiguousarray(inputs["b2"], dtype=np.float32)
    src0 = np.asarray(inputs["src0"]).astype(np.int64)
    src1 = np.asarray(inputs["src1"]).astype(np.int64)
    assert np.abs(b1).max() == 0.0, "nonzero b1 handled by numpy fallback"

    nu_max = max(
        len(np.unique(src1[c * DST_PC * F:(c + 1) * DST_PC * F]))
        for c in range(NC_N))
    nb1 = (nu_max + BLK - 1) // BLK

    in_maps = []
    for c in range(NC_N):
        idx, S4 = _prep_core(src0, src1, c, nb1)
        in_maps.append({"feat": featb, "w1": w1s, "w2": w2s,
                        "idx": idx, "sel": S4})
    nc = _get_nc(nb1)
    kw = {}
    if trace:
        kw = {"trace": True, "trace_kwargs": trace_kwargs or {}}
    res = run_bass_kernel_spmd(nc, in_maps, list(range(NC_N)), **kw)
    full = np.concatenate(
        [np.ascontiguousarray(res.results[c]["outT"].T) for c in range(NC_N)],
        axis=0)
    full = full + b2[None, :]
    return full, res


def kernel(features, W1, b1, W2, b2, src0, dst0, src1, dst1):
    ins = dict(features=features, W1=W1, b1=b1, W2=W2, b2=b2,
               src0=src0, dst0=dst0, src1=src1, dst1=dst1)
    d0 = np.asarray(dst0); d1 = np.asarray(dst1)
    fixed = (d0 == np.arange(N1 * F) // F).all() and \
            (d1 == np.arange(N2 * F) // F).all() and \
            np.abs(np.asarray(b1)).max() == 0.0
    if not fixed:
        f = np.asarray(features, dtype=np.float64)
        m = f[np.asarray(src0)]
        s = np.zeros((N1, IN_F)); np.add.at(s, d0, m)
        deg = np.bincount(d0, minlength=N1).clip(1)
        h = np.maximum(s / deg[:, None] @ np.asarray(W1) + np.asarray(b1), 0)
        m = h[np.asarray(src1)]
        s = np.zeros((N2, HID)); np.add.at(s, d1, m)
        deg = np.bincount(d1, minlength=N2).clip(1)
        return ((s / deg[:, None]) @ np.asarray(W2) + np.asarray(b2)
                ).astype(np.float32)
    out, _ = _run(ins)
    return out


if __name__ == "__main__":
    from concourse.timeline_sim import TimelineSim
    ts = TimelineSim(_get_nc(), trace=False)
    ts.simulate()
    print("TimelineSim:", int(ts.time), "ns")
